# revision 1
# baseline (speedup 1.0000x reference)
"""Trainium2 Bass kernel for nn_Loss_net_58110907515037.

Computes the ODE-flow loss (loss, loss1, loss_KL, loss_F) over R=8192
samples, data-parallel over 8 NeuronCores (1024 samples/core).

Integrator: RK4 with call step h=0.1 aligned to the FEM time-cells of
Phi (inside a cell the field is linear in t, so RK4 keeps full order).
Loss/div quadrature uses composite Simpson on the 21-node 0.05 grid;
midpoint nodes reuse the K3-stage state (tanh th3), which is O(h^2)
accurate and validated to ~2e-3 total vs the reference (gate is 2e-2).

Device algorithm (per core, samples packed NCHUNK chunks on partitions):
  - Each RK4 stage j is:  pre_j = A_m @ X0 + M_{j-1} @ th_{j-1} + c~_j
    (two bf16 matmuls into PSUM), th_j = tanh(pre_j + bias) on ACT.
  - M_{j-1} = alpha * A_m @ U_prev folds the `x + alpha*K` update into a
    host-precomputed 30x30 matrix (block-diag expanded host-side).
  - beta (b2) biases are folded into the tanh biases; the materialized
    state X~ differs from the true X by a host-tracked offset delta.
  - div_v sums come from DVE scalar_tensor_tensor accum_out; ||v||^2
    loss sums from ACT Square activations with accum_out.  The loss-node
    products double as comb terms for the X update (no duplicate MMs).
  - Per-core outputs are small stat tiles; the final tiny reduction and
    Simpson weighting happen on the host.
  - All matmul operands are bf16 (validated ~1.7e-3 total rel err vs
    the 2e-2 gate); hidden blocks padded to a 32-row pitch (full PE
    rows + fast weight load).
"""

import numpy as np
import os as _os

# ---- problem constants (must match the reference) ----
T0, T = 0.0, 1.0
M_, L, HID, D = 10, 3, 5, 3
R_TOTAL = 8192
N_CORES = 8
R_CORE = R_TOTAL // N_CORES          # 1024
K30 = 2 * L * HID                    # 30 data rows (2 nz basis fns x L x HID)
KP = 32                              # chunk pitch on partitions (pad 2)

HC = 0.1                             # RK4 call step (one Phi cell)
N_CALLS = 10
N_TANH = 4 * N_CALLS + 1             # 41 tanh evals
N_NODE = 2 * N_CALLS + 1             # 21 quadrature nodes (0.05 grid)
N_M = 21                             # time indices m = t*20, t in stage grid

NCHUNK = int(_os.environ.get('KERNEL_NCHUNK', '4'))
NSPLIT = int(_os.environ.get('KERNEL_NSPLIT', '1'))
F = R_CORE // NCHUNK                 # free dim per core
FH = F // NSPLIT                     # free dim per chain
P120 = NCHUNK * KP                   # partitions for th tiles (padded)
P12 = NCHUNK * D                     # partitions for x tiles
KAP_EVEN = 6.0 / HC                  # v = kappa * vs + beta at start nodes
KAP_ODD = 3.0 / HC                   # ... at midpoint nodes


def _phi(t):
    grid = np.linspace(T0, T, M_ + 1)
    s = t - grid
    hh = (T - T0) / M_
    relu = lambda a: np.maximum(a, 0.0)
    return (M_ / (T - T0)) * (relu(s + hh) - 2.0 * relu(s) + relu(s - hh))


def _time_consts(t, W1, b1, W2, b2, G):
    """Per-time-point padded [30]-row constants (float64).

    Returns A [30,3], c [30], U [3,30], g [30], beta [3].
    Rows are (nz-basis-idx, l, h); all-zero padding if only 1 nz entry.
    """
    ph = _phi(t)
    nz = [i for i in np.argsort(-np.abs(ph))[:2] if ph[i] != 0.0]
    assert 1 <= len(nz) <= 2, (t, ph)
    A = np.zeros((K30, D))
    c = np.zeros(K30)
    U = np.zeros((D, K30))
    g = np.zeros(K30)
    beta = np.zeros(D)
    for ii, i in enumerate(nz):
        for l in range(L):
            r0 = ii * (L * HID) + l * HID
            A[r0:r0 + HID, :] = W1[i, l]            # [HID, D]
            c[r0:r0 + HID] = b1[i, l]
            U[:, r0:r0 + HID] = ph[i] * W2[i, l]    # [D, HID]
            g[r0:r0 + HID] = ph[i] * G[i, l]
        beta += ph[i] * b2[i].sum(axis=0)
    return A, c, U, g, beta


def _prep(W1, b1, W2, b2):
    """Host-side fold of all device constants (float64 -> float32 banks)."""
    W1 = np.asarray(W1, np.float64)
    b1 = np.asarray(b1, np.float64)
    W2 = np.asarray(W2, np.float64)
    b2 = np.asarray(b2, np.float64)
    G = np.einsum('ildh,ilhd->ilh', W2, W1)   # [11, L, HID]

    tc = {}

    def tcs(m):
        # time index m = t * 20, t in {0, 0.05, ..., 1.0}
        if m not in tc:
            tc[m] = _time_consts(m / 20.0, W1, b1, W2, b2, G)
        return tc[m]

    h = HC
    Ab = np.zeros((P12, N_M * P120), np.float32)      # block-diag A^T per m
    Mb = np.zeros((P120, 6 * N_CALLS * P120), np.float32)  # expanded M^T bank
    cb = np.zeros((P120, N_TANH), np.float32)         # tanh biases
    gb = np.zeros((P120, N_NODE), np.float32)         # div g vectors
    Ub = np.zeros((P120, (3 * N_CALLS + 1) * P12), np.float32)  # gamma*U^T
    bb = np.zeros((P12, N_NODE), np.float32)          # loss stt scalars
    beta2 = np.zeros(N_NODE)                          # sum_d beta_d^2 per p
    gsum = np.zeros(N_NODE)                           # sum_h g_h per q
    kap2 = np.zeros(N_NODE)                           # per-node kappa^2

    def put_A(m, A):
        for u in range(NCHUNK):
            Ab[D * u:D * u + D, P120 * m + KP * u:P120 * m + KP * u + K30] = \
                A.T.astype(np.float32)

    def put_M(e, Mmat):
        MT = Mmat.T.astype(np.float32)
        for u in range(NCHUNK):
            Mb[KP * u:KP * u + K30,
               P120 * e + KP * u:P120 * e + KP * u + K30] = MT

    def put_U(b, U, gamma):
        for u in range(NCHUNK):
            Ub[KP * u:KP * u + K30, P12 * b + D * u:P12 * b + D * u + D] = \
                (gamma * U).T.astype(np.float32)

    def padv(vec):
        return np.concatenate([vec, np.zeros(KP - K30)])

    def put_c(e, cvec):
        cb[:, e] = np.tile(padv(cvec), NCHUNK).astype(np.float32)

    gam = (h / 6.0, h / 3.0, h / 6.0)   # gamma for (th1, th2&th3, th4)

    delta = np.zeros(D)
    for call in range(N_CALLS):
        m1 = 2 * call
        A1, c1, U1, g1, be1 = tcs(m1)
        A2, c2, U2, g2, be2 = tcs(m1 + 1)
        A3, c3, U3, g3, be3 = tcs(m1 + 2)
        put_A(m1, A1)
        put_A(m1 + 1, A2)
        if call == N_CALLS - 1:
            put_A(m1 + 2, A3)
        # tanh biases (fold delta and beta terms)
        put_c(4 * call + 0, c1 + A1 @ delta)
        put_c(4 * call + 1, c2 + A2 @ (delta + (h / 2) * be1))
        put_c(4 * call + 2, c2 + A2 @ (delta + (h / 2) * be2))
        put_c(4 * call + 3, c3 + A3 @ (delta + h * be2))
        # M matrices (stored transposed, block-diag expanded)
        put_M(6 * call + 0, (h / 2) * A2 @ U1)
        put_M(6 * call + 1, (h / 2) * A2 @ U2)
        put_M(6 * call + 2, h * A3 @ U2)
        # boundary: pre1(next) = A3 @ X~ + sum_j gamma_j (A3 @ U_j) th_j
        put_M(6 * call + 3, (h / 6) * A3 @ U1)
        put_M(6 * call + 4, (h / 3) * A3 @ U2)
        put_M(6 * call + 5, (h / 6) * A3 @ U3)
        # U bank (comb & loss)
        put_U(3 * call + 0, U1, gam[0])
        put_U(3 * call + 1, U2, gam[1])
        put_U(3 * call + 2, U3, gam[2])
        # start node 2*call (th1)
        q = 2 * call
        gb[:, q] = np.tile(padv(g1), NCHUNK).astype(np.float32)
        gsum[q] = g1.sum()
        bb[:, q] = np.tile((1.0 / KAP_EVEN) * be1, NCHUNK).astype(np.float32)
        beta2[q] = (be1 ** 2).sum()
        kap2[q] = KAP_EVEN ** 2
        # midpoint node 2*call+1 (th3)
        q = 2 * call + 1
        gb[:, q] = np.tile(padv(g2), NCHUNK).astype(np.float32)
        gsum[q] = g2.sum()
        bb[:, q] = np.tile((1.0 / KAP_ODD) * be2, NCHUNK).astype(np.float32)
        beta2[q] = (be2 ** 2).sum()
        kap2[q] = KAP_ODD ** 2
        delta = delta + (h / 6.0) * (be1 + 4.0 * be2 + be3)

    # final node at t = 1.0
    Af, cf, Uf, gf, bef = tcs(2 * N_CALLS)
    put_c(4 * N_CALLS, cf + Af @ delta)
    put_U(3 * N_CALLS, Uf, gam[0])
    q = N_NODE - 1
    gb[:, q] = np.tile(padv(gf), NCHUNK).astype(np.float32)
    gsum[q] = gf.sum()
    bb[:, q] = np.tile((1.0 / KAP_EVEN) * bef, NCHUNK).astype(np.float32)
    beta2[q] = (bef ** 2).sum()
    kap2[q] = KAP_EVEN ** 2

    dN = delta - 1.0                                   # MEAN1 = 1.0
    dn2 = np.tile(2.0 * dN, NCHUNK).astype(np.float32).reshape(P12, 1)

    # composite Simpson weights on the 21-node 0.05 grid
    w1 = np.ones(N_NODE)
    w1[1:-1:2] = 4.0
    w1[2:-1:2] = 2.0
    wq = w1 * (-(h / 6.0))

    return dict(Ab=Ab, Mb=Mb, cb=cb, gb=gb, Ub=Ub, bb=bb, dn2=dn2,
                beta2=beta2, gsum=gsum, w1=w1, wq=wq, dN=dN, kap2=kap2)


def _combine(prep, dstat, lstat, qstat):
    """Final scalar combine from stat sums (already summed over cores and
    partitions): dstat [21], lstat [21], qstat [2]."""
    R = float(R_TOTAL)
    vsq = prep['kap2'] * lstat        # ||v||^2 per node (Square-bias form)
    loss1 = HC / (6.0 * R) * float(np.dot(prep['w1'], vsq))
    divC = float(np.dot(prep['wq'], prep['gsum'] - dstat / R))
    q0_mean = qstat[0] / R
    qN_mean = (qstat[1] + R * float((prep['dN'] ** 2).sum())) / R
    loss_KL = -0.5 * q0_mean + divC + 0.5 * qN_mean
    loss_F = 0.0
    loss = loss1 + loss_KL + loss_F
    f32 = np.float32
    return f32(loss), f32(loss1), f32(loss_KL), f32(loss_F)


def _pack_x(x_core):
    """[R_CORE, D] -> [P12, F] packed (chunk-major partitions), bf16."""
    import ml_dtypes
    return np.ascontiguousarray(
        x_core.reshape(NCHUNK, F, D).transpose(0, 2, 1).reshape(P12, F)
    ).astype(ml_dtypes.bfloat16)


def _model_core(prep, xp):
    """Numpy float32 simulation of the device program for one core.

    xp: [P12, F]. Returns dstat [P120, 21], lstat [P12, 21], qstat [P12, 2].
    """
    f32 = np.float32
    Ab, Mb, cb, gb, Ub, bb, dn2 = (prep[k] for k in
                                   ('Ab', 'Mb', 'cb', 'gb', 'Ub', 'bb', 'dn2'))
    dstat = np.zeros((P120, N_NODE), f32)
    lstat = np.zeros((P12, N_NODE), f32)
    qstat = np.zeros((P12, 2), f32)

    def mm(lhsT, rhs):
        return (lhsT.T.astype(f32) @ rhs.astype(f32)).astype(f32)

    X = xp.astype(f32)
    qstat[:, 0] = ((X + 0.0) * X).sum(axis=1)

    def A_l(m):
        return Ab[:, P120 * m:P120 * (m + 1)]

    def U_l(b):
        return Ub[:, P12 * b:P12 * (b + 1)]

    def M_l(e):
        return Mb[:, P120 * e:P120 * (e + 1)]

    def div_stt(th, q):
        dstat[:, q] = ((th * gb[:, q:q + 1]) * th).sum(axis=1)

    def loss_stt(vs, p):
        lstat[:, p] = ((vs + bb[:, p:p + 1]) ** 2).sum(axis=1)

    pre1 = None
    for call in range(N_CALLS):
        m1 = 2 * call
        e6 = 6 * call
        if call == 0:
            pre1 = mm(A_l(m1), X)
        th1 = np.tanh(pre1 + cb[:, 4 * call:4 * call + 1])
        div_stt(th1, 2 * call)
        loss_stt(mm(U_l(3 * call), th1), 2 * call)
        th2 = np.tanh(mm(A_l(m1 + 1), X) + mm(M_l(e6 + 0), th1)
                      + cb[:, 4 * call + 1:4 * call + 2])
        th3 = np.tanh(mm(A_l(m1 + 1), X) + mm(M_l(e6 + 1), th2)
                      + cb[:, 4 * call + 2:4 * call + 3])
        div_stt(th3, 2 * call + 1)
        loss_stt(mm(U_l(3 * call + 1), th3), 2 * call + 1)
        th4 = np.tanh(mm(A_l(m1 + 2), X) + mm(M_l(e6 + 2), th3)
                      + cb[:, 4 * call + 3:4 * call + 4])
        pre1 = (mm(A_l(m1 + 2), X) + mm(M_l(e6 + 3), th1)
                + mm(M_l(e6 + 4), th2) + mm(M_l(e6 + 4), th3)
                + mm(M_l(e6 + 5), th4))
        comb = (mm(U_l(3 * call), th1) + mm(U_l(3 * call + 1), th2)
                + mm(U_l(3 * call + 1), th3) + mm(U_l(3 * call + 2), th4))
        X = (X + comb).astype(f32)

    thf = np.tanh(pre1 + cb[:, 4 * N_CALLS:4 * N_CALLS + 1])
    div_stt(thf, N_NODE - 1)
    loss_stt(mm(U_l(3 * N_CALLS), thf), N_NODE - 1)
    qstat[:, 1] = ((X + dn2[:, 0:1]) * X).sum(axis=1)
    return dstat, lstat, qstat


def _run_model(prep, x):
    dstat = np.zeros(N_NODE)
    lstat = np.zeros(N_NODE)
    qstat = np.zeros(2)
    for c in range(N_CORES):
        xp = _pack_x(np.asarray(x[c * R_CORE:(c + 1) * R_CORE], np.float32))
        d, l, q = _model_core(prep, xp)
        dstat += d.sum(axis=0)
        lstat += l.sum(axis=0)
        qstat += q.sum(axis=0)
    return _combine(prep, dstat, lstat, qstat)


def kernel(x, W1, b1, W2, b2):
    prep = _prep(W1, b1, W2, b2)
    if _os.environ.get('KERNEL_NUMPY_MODEL'):
        return _run_model(prep, np.asarray(x, np.float32))
    dstat, lstat, qstat = _run_device(prep, np.asarray(x, np.float32))
    return _combine(prep, dstat, lstat, qstat)


_BASS_CACHE = {}


def _build_bass():
    """Build the Bass/Tile program (shape-only; constants arrive as inputs).

    NSPLIT independent chains run staggered so ACT/PE/DVE overlap; with
    NSPLIT=1 the free dim stays 256 so fp32r matmuls run at full rate.
    """
    import concourse.mybir as mybir
    from concourse import tile, bacc

    f32 = mybir.dt.float32
    bf16 = mybir.dt.bfloat16
    AF = mybir.ActivationFunctionType
    OP = mybir.AluOpType

    nc = bacc.Bacc(None, target_bir_lowering=False)
    dp = nc.declare_dram_parameter
    xp_d = dp("xp", [P12, F], bf16, isOutput=False)
    Ab_d = dp("Ab", [P12, N_M * P120], bf16, isOutput=False)
    Mb_d = dp("Mb", [P120, 6 * N_CALLS * P120], bf16, isOutput=False)
    cb_d = dp("cb", [P120, N_TANH], f32, isOutput=False)
    gb_d = dp("gb", [P120, N_NODE], f32, isOutput=False)
    Ub_d = dp("Ub", [P120, (3 * N_CALLS + 1) * P12], bf16, isOutput=False)
    bb_d = dp("bb", [P12, N_NODE], f32, isOutput=False)
    dn2_d = dp("dn2", [P12, 1], f32, isOutput=False)
    stat_d = dp("stat", [P120, (2 * N_NODE + 2) * NSPLIT], f32, isOutput=True)

    with tile.TileContext(nc) as tc:
        with (
            tc.tile_pool(name="const", bufs=1) as cpool,
            tc.tile_pool(name="state", bufs=2) as xpool,
            tc.tile_pool(name="th", bufs=2) as thpool,
            tc.tile_pool(name="scr", bufs=2) as spool,
            tc.tile_pool(name="pre", bufs=4, space="PSUM") as prepool,
            tc.tile_pool(name="acc", bufs=2, space="PSUM") as accpool,
        ):
            xp_t = [None] * NSPLIT
            Ab_t = cpool.tile([P12, N_M * P120], bf16)
            Mb_t = cpool.tile([P120, 6 * N_CALLS * P120], bf16)
            cb_t = cpool.tile([P120, N_TANH], f32)
            gb_t = cpool.tile([P120, N_NODE], f32)
            Ub_t = cpool.tile([P120, (3 * N_CALLS + 1) * P12], bf16)
            bb_t = cpool.tile([P12, N_NODE], f32)
            dn2_t = cpool.tile([P12, 1], f32)
            stat_t = cpool.tile([P120, (2 * N_NODE + 2) * NSPLIT], f32)
            dstat_t = stat_t[:, :N_NODE * NSPLIT]
            lstat_t = stat_t[:P12, N_NODE * NSPLIT:2 * N_NODE * NSPLIT]
            qstat_t = stat_t[:P12, 2 * N_NODE * NSPLIT:]

            # call-0-critical transfers first: descriptor-gen on SP is
            # serial AND each DMA queue drains in order, so both emission
            # order and transfer size matter.  xp (6 KB) must not queue
            # behind the 1.7 MB Mb bank.
            for _h in range(NSPLIT):
                _Xh = xpool.tile([P12, FH], bf16, name=f"X{_h}", tag=f"X{_h}")
                nc.sync.dma_start(out=_Xh[:],
                                  in_=xp_d[:, FH * _h:FH * (_h + 1)])
                xp_t[_h] = _Xh
            nc.sync.dma_start(out=Ab_t[:, :6 * P120], in_=Ab_d[:, :6 * P120])
            nc.sync.dma_start(out=cb_t[:], in_=cb_d[:])
            nc.sync.dma_start(out=Mb_t[:, :12 * P120], in_=Mb_d[:, :12 * P120])
            nc.sync.dma_start(out=Ub_t[:], in_=Ub_d[:])
            nc.sync.dma_start(out=gb_t[:], in_=gb_d[:])
            nc.sync.dma_start(out=bb_t[:], in_=bb_d[:])
            nc.sync.dma_start(out=dn2_t[:], in_=dn2_d[:])
            nc.sync.dma_start(out=Ab_t[:, 6 * P120:], in_=Ab_d[:, 6 * P120:])
            for e0 in range(12, 6 * N_CALLS, 24):
                e1 = min(e0 + 24, 6 * N_CALLS)
                nc.sync.dma_start(out=Mb_t[:, P120 * e0:P120 * e1],
                                  in_=Mb_d[:, P120 * e0:P120 * e1])

            def A_ap(m):
                return Ab_t[:, P120 * m:P120 * (m + 1)]

            def M_ap(e):
                return Mb_t[:, P120 * e:P120 * (e + 1)]

            def U_ap(b):
                return Ub_t[:, P12 * b:P12 * (b + 1)]

            X = list(xp_t)
            for h in range(NSPLIT):
                scr12 = spool.tile([P12, FH], f32, name="scr12q", tag="s12q")
                nc.vector.scalar_tensor_tensor(
                    out=scr12[:], in0=X[h][:], scalar=0.0,
                    in1=X[h][:], op0=OP.add, op1=OP.mult,
                    accum_out=qstat_t[:, 0 * NSPLIT + h:0 * NSPLIT + h + 1])

            def div_stt(h, th, q):
                scr = spool.tile([P120, FH], bf16, name="scr", tag="scr")
                col = q * NSPLIT + h
                nc.vector.scalar_tensor_tensor(
                    out=scr[:], in0=th[:], scalar=gb_t[:, q:q + 1],
                    in1=th[:], op0=OP.mult, op1=OP.mult,
                    accum_out=dstat_t[:, col:col + 1])

            def loss_mm(h, th, b):
                # the node's vs = gamma*U@th is also a comb term: compute it
                # once into its own PSUM bank, reused by the X update.
                vps = accpool.tile([P12, FH], f32, name="vps", tag="vps",
                                   bufs=3)
                nc.tensor.matmul(vps[:], U_ap(b), th[:],
                                 start=True, stop=True)
                return vps

            def loss_red(h, vps, p, eng):
                # sum_r (vs + beta/kappa)^2; the beta^2 excess cancels in the
                # host combine (vsq = kappa^2 * lstat)
                col = p * NSPLIT + h
                if eng == 'act':
                    vsb = spool.tile([P12, FH], bf16, name="vsb", tag="s12")
                    nc.scalar.activation(vsb[:], vps[:], AF.Square,
                                         bias=bb_t[:, p:p + 1],
                                         accum_out=lstat_t[:, col:col + 1])
                else:
                    w = spool.tile([P12, FH], bf16, name="w", tag="s12")
                    nc.vector.tensor_scalar_add(w[:], vps[:],
                                                bb_t[:, p:p + 1])
                    scr12 = spool.tile([P12, FH], bf16, name="scr12",
                                       tag="s12")
                    nc.vector.scalar_tensor_tensor(
                        out=scr12[:], in0=w[:], scalar=1.0,
                        in1=w[:], op0=OP.mult, op1=OP.mult,
                        accum_out=lstat_t[:, col:col + 1])

            def a_mm(h, m, last):
                pre = prepool.tile([P120, FH], f32, name="pre", tag="pre")
                nc.tensor.matmul(pre[:], A_ap(m), X[h][:],
                                 start=True, stop=last)
                return pre

            def m_mm(pre, e, th_prev):
                nc.tensor.matmul(pre[:], M_ap(e), th_prev[:],
                                 start=False, stop=True)

            def tanh_of(h, pre, e):
                th = thpool.tile([P120, FH], bf16, name=f"th{e % 4}_{h}",
                                 tag=f"th{e % 4}_{h}", bufs=3)
                nc.scalar.activation(th[:], pre[:], AF.Tanh,
                                     bias=cb_t[:, e:e + 1])
                return th

            th1 = [None] * NSPLIT
            th2 = [None] * NSPLIT
            th3 = [None] * NSPLIT
            th4 = [None] * NSPLIT
            pre_t = {}
            comb = [None] * NSPLIT
            vps1 = [None] * NSPLIT
            t1 = [None] * NSPLIT
            t2 = [None] * NSPLIT
            pre1_next = [None] * NSPLIT
            pending_red = []
            for call in range(N_CALLS):
                m1 = 2 * call
                e0 = 4 * call
                e6 = 6 * call
                for h in range(NSPLIT):
                    if call == 0:
                        pre_t[(h, 1)] = a_mm(h, m1, True)
                    else:
                        pre_t[(h, 1)] = pre1_next[h]
                for h in range(NSPLIT):
                    th1[h] = tanh_of(h, pre_t[(h, 1)], e0)
                for (ph, pv, pp) in pending_red:
                    loss_red(ph, pv, pp, 'act')
                pending_red = []
                # stage-2 path first: its A-part waits on the X update
                # (the binding cycle), so it must lead the PE queue
                for h in range(NSPLIT):
                    pre_t[(h, 2)] = a_mm(h, m1 + 1, False)
                for h in range(NSPLIT):
                    m_mm(pre_t[(h, 2)], e6 + 0, th1[h])
                for h in range(NSPLIT):
                    pre1_next[h] = a_mm(h, m1 + 2, False)
                for h in range(NSPLIT):
                    nc.tensor.matmul(pre1_next[h][:], M_ap(e6 + 3),
                                     th1[h][:], start=False, stop=False)
                for h in range(NSPLIT):
                    pre_t[(h, 3)] = a_mm(h, m1 + 1, False)
                for h in range(NSPLIT):
                    vps1[h] = loss_mm(h, th1[h], 3 * call)
                for h in range(NSPLIT):
                    th2[h] = tanh_of(h, pre_t[(h, 2)], e0 + 1)
                for h in range(NSPLIT):
                    div_stt(h, th1[h], 2 * call)
                    loss_red(h, vps1[h], 2 * call, 'act')
                    t1[h] = spool.tile([P12, FH], f32, name="t1", tag="t12")
                    nc.vector.tensor_add(t1[h][:], vps1[h][:], X[h][:])
                for h in range(NSPLIT):
                    m_mm(pre_t[(h, 3)], e6 + 1, th2[h])
                for h in range(NSPLIT):
                    nc.tensor.matmul(pre1_next[h][:], M_ap(e6 + 4),
                                     th2[h][:], start=False, stop=False)
                    comb[h] = accpool.tile([P12, FH], f32, name="comb",
                                           tag="comb", bufs=1)
                    nc.tensor.matmul(comb[h][:], U_ap(3 * call + 1),
                                     th2[h][:], start=True, stop=False)
                for h in range(NSPLIT):
                    th3[h] = tanh_of(h, pre_t[(h, 3)], e0 + 2)
                for h in range(NSPLIT):
                    pre_t[(h, 4)] = a_mm(h, m1 + 2, False)
                for h in range(NSPLIT):
                    m_mm(pre_t[(h, 4)], e6 + 2, th3[h])
                for h in range(NSPLIT):
                    nc.tensor.matmul(pre1_next[h][:], M_ap(e6 + 4),
                                     th3[h][:], start=False, stop=False)
                vps3 = [None] * NSPLIT
                for h in range(NSPLIT):
                    vps3[h] = loss_mm(h, th3[h], 3 * call + 1)
                for h in range(NSPLIT):
                    th4[h] = tanh_of(h, pre_t[(h, 4)], e0 + 3)
                for h in range(NSPLIT):
                    div_stt(h, th3[h], 2 * call + 1)
                    t2[h] = spool.tile([P12, FH], f32, name="t2", tag="t12")
                    nc.vector.tensor_add(t2[h][:], vps3[h][:], t1[h][:])
                    pending_red.append((h, vps3[h], 2 * call + 1))
                for h in range(NSPLIT):
                    nc.tensor.matmul(comb[h][:], U_ap(3 * call + 2),
                                     th4[h][:], start=False, stop=True)
                for h in range(NSPLIT):
                    Xn = xpool.tile([P12, FH], bf16, name=f"X{h}",
                                    tag=f"X{h}")
                    nc.vector.tensor_add(Xn[:], comb[h][:], t2[h][:])
                    X[h] = Xn
                for h in range(NSPLIT):
                    nc.tensor.matmul(pre1_next[h][:], M_ap(e6 + 5),
                                     th4[h][:], start=False, stop=True)

            # final extra eval at t = 1.0: pre1_next already holds it
            for h in range(NSPLIT):
                scr12b = spool.tile([P12, FH], f32, name="scr12q",
                                    tag="s12q")
                col = 1 * NSPLIT + h
                nc.vector.scalar_tensor_tensor(
                    out=scr12b[:], in0=X[h][:], scalar=dn2_t[:, 0:1],
                    in1=X[h][:], op0=OP.add, op1=OP.mult,
                    accum_out=qstat_t[:, col:col + 1])
                thf = tanh_of(h, pre1_next[h], 4 * N_CALLS)
                for (ph, pv, pp) in pending_red:
                    loss_red(ph, pv, pp, 'act')
                pending_red = []
                div_stt(h, thf, N_NODE - 1)
                vpsf = loss_mm(h, thf, 3 * N_CALLS)
                loss_red(h, vpsf, N_NODE - 1, 'dve')

            nc.sync.dma_start(out=stat_d[:], in_=stat_t[:])
    nc.compile()
    return nc


def _const_map(prep):
    import ml_dtypes
    b = ml_dtypes.bfloat16
    return dict(Ab=prep['Ab'].astype(b), Mb=prep['Mb'].astype(b),
                cb=prep['cb'], gb=prep['gb'], Ub=prep['Ub'].astype(b),
                bb=prep['bb'], dn2=prep['dn2'])


def _run_device(prep, x):
    from concourse.bass_utils import run_bass_kernel_spmd
    if 'nc' not in _BASS_CACHE:
        _BASS_CACHE['nc'] = _build_bass()
    nc = _BASS_CACHE['nc']
    consts = _const_map(prep)
    in_maps = []
    for c in range(N_CORES):
        m = dict(consts)
        m['xp'] = _pack_x(x[c * R_CORE:(c + 1) * R_CORE])
        in_maps.append(m)
    trace = bool(_os.environ.get('KERNEL_TRACE'))
    res = run_bass_kernel_spmd(nc, in_maps, list(range(N_CORES)),
                               trace=trace)
    _BASS_CACHE['last_result'] = res
    dstat = np.zeros(N_NODE)
    lstat = np.zeros(N_NODE)
    qstat = np.zeros(2)
    for c in range(N_CORES):
        st = res.results[c]['stat'].astype(np.float64)
        dstat += st[:, :N_NODE * NSPLIT].sum(axis=0) \
            .reshape(N_NODE, NSPLIT).sum(axis=1)
        lstat += st[:P12, N_NODE * NSPLIT:2 * N_NODE * NSPLIT].sum(axis=0) \
            .reshape(N_NODE, NSPLIT).sum(axis=1)
        qstat += st[:P12, 2 * N_NODE * NSPLIT:].sum(axis=0) \
            .reshape(2, NSPLIT).sum(axis=1)
    return dstat, lstat, qstat



# revision 37
# speedup vs baseline: 1.1175x; 1.1175x over previous
"""Trainium2 Bass kernel for nn_Loss_net_58110907515037.

Computes the ODE-flow loss (loss, loss1, loss_KL, loss_F) over R=8192
samples, data-parallel over 8 NeuronCores (1024 samples/core).

Integrator: classic RK3 (Kutta) with step h=0.1 aligned to the FEM
time-cells of Phi.  Float64 study: RK3 h=0.1 truncation is ~4.1e-3 vs
the 2e-2 gate (RK4 h=0.1 was 1.8e-3); RK3 drops the serial tanh chain
from 41 to 31 and the matmul count from ~15 to ~10 per call, and its
stage values double as the quadrature nodes (k1 = start node value,
k2 = midpoint value), so no extra loss matmuls are needed.

Key structural points (per core, NCHUNK=4 sample chunks on partitions):
  - Grid-node stages (t on the 0.1 grid) have only ONE nonzero Phi
    basis -> 15 live hidden rows/chunk (60 partitions); midpoints have
    two -> 30 rows/chunk (120 partitions).  All weights are shrunk
    accordingly (less LDWEIGHTS + PSUM).
  - pre3 (stage-3 preact) and pre1' (next call's stage-1 preact) share
    their A3@X part: both live stacked in ONE [120,F] PSUM tile, fed by
    single matmuls with [W | W'] stacked weights.
  - X update folds beta terms into host-tracked delta (biases adjust);
    vps PSUM tiles double as loss-node values via the kappa trick.
  - Loss reductions run on GpSimd, div reductions + X update on DVE,
    so ACT does nothing but the 31 critical-path tanh ops.
  - sum_r(v+b)^2 is computed as one stt op: (vps + 2b')*vps summed,
    with the b'^2 correction applied host-side.
"""

import numpy as np
import os as _os

# ---- problem constants (must match the reference) ----
T0, T = 0.0, 1.0
M_, L, HID, D = 10, 3, 5, 3
R_TOTAL = 8192
N_CORES = 8
R_CORE = R_TOTAL // N_CORES          # 1024
NCHUNK = 4
F = R_CORE // NCHUNK                 # 256 free dim
P12 = NCHUNK * D                     # 12 partitions for x/vps tiles
P60 = NCHUNK * 15                    # grid-node th partitions
P120 = NCHUNK * 30                   # midpoint th partitions

HC = 0.1                             # RK3 step (one Phi cell)
N_CALLS = 10
N_NODE = 2 * N_CALLS + 1             # 21 quadrature nodes (0.05 grid)
KAP_E = 6.0 / HC                     # v = kap*vps + be at grid nodes
KAP_O = 3.0 / (2.0 * HC)             # ... at midpoints (gamma = 2h/3)

OFF1 = 64                                      # pre1' partition base (32-al.)
P124 = OFF1 + P60                              # stacked pre3/pre1' tile rows

# bank column layouts
WB12_C = 60 + (120 + P124) * N_CALLS           # [12, 2500]
WB60_C = (120 + 2 * P124 + 24) * N_CALLS + 12  # [60, 3932]
WB120_C = (P124 + 12) * N_CALLS                # [120, 1360]
FBB_C = 31                                     # bias bank [128, 31]
FBG_C = 21                                     # g bank [120, 21]
FB12_C = 12                                    # loss-bias bank [44, 12]
ST_C = 34                                      # stat out [120, 34]


def _phi(t):
    grid = np.linspace(T0, T, M_ + 1)
    s = t - grid
    hh = (T - T0) / M_
    relu = lambda a: np.maximum(a, 0.0)
    return (M_ / (T - T0)) * (relu(s + hh) - 2.0 * relu(s) + relu(s - hh))


def _tconsts(m, W1, b1, W2, b2, G):
    """Per-time-point constants at t = m/20 (float64).

    Returns A [K,3], c [K], U [3,K], g [K], be [3] with K = 15*len(nz):
    rows (nz-basis-idx, l, h).  K=15 at grid nodes, 30 at midpoints.
    """
    ph = _phi(m / 20.0)
    nz = sorted(i for i in np.argsort(-np.abs(ph))[:2] if abs(ph[i]) > 1e-9)
    assert len(nz) == (1 if m % 2 == 0 else 2), (m, ph)
    K = 15 * len(nz)
    A = np.zeros((K, D))
    c = np.zeros(K)
    U = np.zeros((D, K))
    g = np.zeros(K)
    be = np.zeros(D)
    for ii, i in enumerate(nz):
        for l in range(L):
            r0 = ii * (L * HID) + l * HID
            A[r0:r0 + HID, :] = W1[i, l]
            c[r0:r0 + HID] = b1[i, l]
            U[:, r0:r0 + HID] = ph[i] * W2[i, l]
            g[r0:r0 + HID] = ph[i] * G[i, l]
        be += ph[i] * b2[i].sum(axis=0)
    return A, c, U, g, be


def _expT(mat, pin, pout):
    """Block-diag lhsT expansion: mat [pout,pin] per chunk ->
    [NCHUNK*pin, NCHUNK*pout]."""
    W = np.zeros((NCHUNK * pin, NCHUNK * pout))
    for u in range(NCHUNK):
        W[u * pin:(u + 1) * pin, u * pout:(u + 1) * pout] = mat.T
    return W


def _prep(W1, b1, W2, b2):
    """Host-side fold of all device constants (float64 -> banks)."""
    W1 = np.asarray(W1, np.float64)
    b1 = np.asarray(b1, np.float64)
    W2 = np.asarray(W2, np.float64)
    b2 = np.asarray(b2, np.float64)
    G = np.einsum('ildh,ilhd->ilh', W2, W1)   # [11, L, HID]
    h = HC

    wb12 = np.zeros((P12, WB12_C))
    wb60 = np.zeros((P60, WB60_C))
    wb120 = np.zeros((P120, WB120_C))
    fbB = np.zeros((128, FBB_C), np.float32)
    fbG = np.zeros((P120, FBG_C), np.float32)
    fb12 = np.zeros((44, FB12_C), np.float32)
    beta2 = np.zeros(N_NODE)
    gsum = np.zeros(N_NODE)
    kap2 = np.zeros(N_NODE)

    t4 = lambda v: np.tile(v, NCHUNK)
    delta = np.zeros(D)
    for c in range(N_CALLS):
        A1, c1, U1, g1, be1 = _tconsts(2 * c, W1, b1, W2, b2, G)
        A2, c2, U2, g2, be2 = _tconsts(2 * c + 1, W1, b1, W2, b2, G)
        A3, c3, U3, g3, be3 = _tconsts(2 * c + 2, W1, b1, W2, b2, G)
        z4 = lambda p: np.zeros((p, OFF1 - P60))
        if c == 0:
            wb12[:, 0:60] = _expT(A1, D, 15)
            fbB[:P60, 10] = t4(c1 + A1 @ delta)        # call-0 th1 bias
        b0 = 60 + (120 + P124) * c
        wb12[:, b0:b0 + 120] = _expT(A2, D, 30)
        wb12[:, b0 + 120:b0 + 120 + P124] = np.hstack(
            [_expT(A3, D, 15), z4(P12), _expT(A3, D, 15)])
        b0 = (120 + 2 * P124 + 24) * c
        wb60[:, b0:b0 + 120] = _expT((h / 2) * A2 @ U1, 15, 30)
        wb60[:, b0 + 120:b0 + 120 + P124] = np.hstack(
            [_expT(-h * A3 @ U1, 15, 15), z4(P60),
             _expT((h / 6) * A3 @ U1, 15, 15)])
        b1_ = b0 + 120 + P124
        wb60[:, b1_:b1_ + P124] = np.hstack(
            [np.zeros((P60, OFF1)), _expT((h / 6) * A3 @ U3, 15, 15)])
        wb60[:, b1_ + P124:b1_ + P124 + 12] = _expT((h / 6) * U1, 15, D)
        wb60[:, b1_ + P124 + 12:b1_ + P124 + 24] = _expT((h / 6) * U3, 15, D)
        b0 = (P124 + 12) * c
        wb120[:, b0:b0 + P124] = np.hstack(
            [_expT(2 * h * A3 @ U2, 30, 15), z4(P120),
             _expT((2 * h / 3) * A3 @ U2, 30, 15)])
        wb120[:, b0 + P124:b0 + P124 + 12] = _expT((2 * h / 3) * U2, 30, D)
        # biases
        fbB[:P120, c] = t4(c2 + A2 @ (delta + (h / 2) * be1))      # th2
        fbB[:P60, 21 + c] = t4(c3 + A3 @ (delta - h * be1 + 2 * h * be2))
        delta = delta + (h / 6.0) * (be1 + 4.0 * be2 + be3)
        fbB[OFF1:P124, 11 + c] = t4(c3 + A3 @ delta)   # next th1 (rows 64+)
        # node data
        fbG[:P60, c] = t4(g1)
        fbG[:, 11 + c] = t4(g2)
        gsum[2 * c] = g1.sum()
        gsum[2 * c + 1] = g2.sum()
        fb12[:P12, c] = t4(be1 / KAP_E)
        fb12[32:, c] = t4(be2 / KAP_O)
        beta2[2 * c] = (be1 ** 2).sum()
        beta2[2 * c + 1] = (be2 ** 2).sum()
        kap2[2 * c] = KAP_E ** 2
        kap2[2 * c + 1] = KAP_O ** 2

    # final node at t = 1.0 (bias for thf already set as call-9 "next th1")
    Af, cf, Uf, gf, bef = _tconsts(2 * N_CALLS, W1, b1, W2, b2, G)
    wb60[:, WB60_C - 12:] = _expT((h / 6) * Uf, 15, D)
    q = N_NODE - 1
    fbG[:P60, 10] = t4(gf)
    gsum[q] = gf.sum()
    fb12[:P12, 10] = t4(bef / KAP_E)
    beta2[q] = (bef ** 2).sum()
    kap2[q] = KAP_E ** 2

    dN = delta - 1.0                                   # MEAN1 = 1.0
    fb12[:P12, 11] = t4(2.0 * dN)

    w1 = np.ones(N_NODE)
    w1[1:-1:2] = 4.0
    w1[2:-1:2] = 2.0
    wq = w1 * (-(h / 6.0))

    return dict(wb12=wb12, wb60=wb60, wb120=wb120, fbB=fbB, fbG=fbG,
                fb12=fb12, beta2=beta2, gsum=gsum, w1=w1, wq=wq, dN=dN,
                kap2=kap2)


def _combine(prep, dstat, lstat, qstat):
    """Final scalar combine from stat sums (already summed over cores and
    partitions): dstat [21], lstat [21], qstat [2]."""
    R = float(R_TOTAL)
    vsq = prep['kap2'] * lstat                       # sum_r ||v||^2 per node
    loss1 = HC / (6.0 * R) * float(np.dot(prep['w1'], vsq))
    divC = float(np.dot(prep['wq'], prep['gsum'] - dstat / R))
    q0_mean = qstat[0] / R
    qN_mean = (qstat[1] + R * float((prep['dN'] ** 2).sum())) / R
    loss_KL = -0.5 * q0_mean + divC + 0.5 * qN_mean
    loss_F = 0.0
    loss = loss1 + loss_KL + loss_F
    f32 = np.float32
    return f32(loss), f32(loss1), f32(loss_KL), f32(loss_F)


def _pack_x(x_core):
    """[R_CORE, D] -> [P12, F] packed (chunk-major partitions), bf16."""
    import ml_dtypes
    return np.ascontiguousarray(
        x_core.reshape(NCHUNK, F, D).transpose(0, 2, 1).reshape(P12, F)
    ).astype(ml_dtypes.bfloat16)


def _model_core(prep, xp):
    """Numpy f32 mirror of the device program for one core.

    xp: [P12, F] bf16.  Returns dstat [21], lstat [21], qstat [2]
    (summed over partitions)."""
    import ml_dtypes
    bf = ml_dtypes.bfloat16
    f32 = np.float32
    wb12, wb60, wb120 = (prep[k].astype(bf).astype(f32)
                         for k in ('wb12', 'wb60', 'wb120'))
    fbB, fbG, fb12 = prep['fbB'], prep['fbG'], prep['fb12']
    dstat = np.zeros(N_NODE)
    lstat = np.zeros(N_NODE)
    qstat = np.zeros(2)

    def mm(lhsT, rhs):
        return (lhsT.T @ rhs.astype(bf).astype(f32)).astype(f32)

    X = xp.astype(f32)
    qstat[0] = (X * X).sum()
    tanh = lambda p, b: np.tanh(p + b[:, None]).astype(bf).astype(f32)
    pre31p = None
    for c in range(N_CALLS):
        if c == 0:
            pre1 = mm(wb12[:, 0:60], X)
            th1 = tanh(pre1, fbB[:P60, 10])
        else:
            th1 = tanh(pre31p[OFF1:], fbB[OFF1:P124, 10 + c])
        dstat[2 * c] = ((th1 * fbG[:P60, c:c + 1]) * th1).sum()
        b0 = 60 + (120 + P124) * c
        a2 = mm(wb12[:, b0:b0 + 120], X)
        a33 = mm(wb12[:, b0 + 120:b0 + 120 + P124], X)
        b0 = (120 + 2 * P124 + 24) * c
        b1_ = b0 + 120 + P124
        vps1 = mm(wb60[:, b1_ + P124:b1_ + P124 + 12], th1)
        lstat[2 * c] = ((vps1 + fb12[:P12, c:c + 1]) ** 2).sum()
        th2 = tanh(a2 + mm(wb60[:, b0:b0 + 120], th1), fbB[:P120, c])
        dstat[2 * c + 1] = ((th2 * fbG[:, 11 + c:12 + c]) * th2).sum()
        bw = (P124 + 12) * c
        pre31 = a33 + mm(wb60[:, b0 + 120:b0 + 120 + P124], th1) \
            + mm(wb120[:, bw:bw + P124], th2)
        vps23 = mm(wb120[:, bw + P124:bw + P124 + 12], th2)
        lstat[2 * c + 1] = ((vps23 + fb12[32:, c:c + 1]) ** 2).sum()
        th3 = tanh(pre31[:P60], fbB[:P60, 21 + c])
        pre31 = pre31 + mm(wb60[:, b1_:b1_ + P124], th3)
        vps23 = vps23 + mm(wb60[:, b1_ + P124 + 12:b1_ + P124 + 24], th3)
        t1 = X + vps1
        X = (t1 + vps23).astype(bf).astype(f32)
        pre31p = pre31
    thf = tanh(pre31p[OFF1:], fbB[OFF1:P124, 20])
    dstat[N_NODE - 1] = ((thf * fbG[:P60, 10:11]) * thf).sum()
    vpsf = mm(wb60[:, WB60_C - 12:], thf)
    q = N_NODE - 1
    lstat[q] = ((vpsf + fb12[:P12, 10:11]) ** 2).sum()
    qstat[1] = ((X + fb12[:P12, 11:12]) * X).sum()
    return dstat, lstat, qstat


def _run_model(prep, x):
    dstat = np.zeros(N_NODE)
    lstat = np.zeros(N_NODE)
    qstat = np.zeros(2)
    for c in range(N_CORES):
        xp = _pack_x(np.asarray(x[c * R_CORE:(c + 1) * R_CORE], np.float32))
        d, l, q = _model_core(prep, xp)
        dstat += d
        lstat += l
        qstat += q
    return _combine(prep, dstat, lstat, qstat)


def kernel(x, W1, b1, W2, b2):
    prep = _prep(W1, b1, W2, b2)
    if _os.environ.get('KERNEL_NUMPY_MODEL'):
        return _run_model(prep, np.asarray(x, np.float32))
    dstat, lstat, qstat = _run_device(prep, np.asarray(x, np.float32))
    return _combine(prep, dstat, lstat, qstat)


_BASS_CACHE = {}


def _build_bass():
    """Build the Bass/Tile program (shape-only; constants arrive as inputs)."""
    import concourse.mybir as mybir
    from concourse import tile, bacc

    f32 = mybir.dt.float32
    bf16 = mybir.dt.bfloat16
    AF = mybir.ActivationFunctionType
    OP = mybir.AluOpType

    nc = bacc.Bacc(None, target_bir_lowering=False)
    dp = nc.declare_dram_parameter
    xp_d = dp("xp", [P12, F], bf16, isOutput=False)
    wb12_d = dp("wb12", [P12, WB12_C], bf16, isOutput=False)
    wb60_d = dp("wb60", [P60, WB60_C], bf16, isOutput=False)
    wb120_d = dp("wb120", [P120, WB120_C], bf16, isOutput=False)
    fbB_d = dp("fbB", [128, FBB_C], f32, isOutput=False)
    fbG_d = dp("fbG", [P120, FBG_C], f32, isOutput=False)
    fb12_d = dp("fb12", [44, FB12_C], f32, isOutput=False)
    stat_d = dp("stat", [P120, ST_C], f32, isOutput=True)

    with tile.TileContext(nc) as tc:
        with (
            tc.tile_pool(name="const", bufs=1) as cpool,
            tc.tile_pool(name="state", bufs=2) as xpool,
            tc.tile_pool(name="th", bufs=2) as thpool,
            tc.tile_pool(name="scr", bufs=2) as spool,
            tc.tile_pool(name="pre", bufs=2, space="PSUM") as prepool,
            tc.tile_pool(name="vps", bufs=1, space="PSUM") as vpool,
        ):
            # ACT table preload: dummy tanh+square on a zeroed scrap tile so
            # the ~1.3us ACT_TABLE_LOAD overlaps the weight DMAs.
            warm = cpool.tile([1, 8], f32)
            nc.gpsimd.memset(warm[:], 0.0)
            warm2 = cpool.tile([1, 8], f32)
            nc.scalar.activation(warm2[:], warm[:], AF.Tanh)
            nc.scalar.activation(warm2[:], warm[:], AF.Square)

            xp_t = cpool.tile([P12, F], bf16)
            wb12_t = cpool.tile([P12, WB12_C], bf16)
            wb60_t = cpool.tile([P60, WB60_C], bf16)
            wb120_t = cpool.tile([P120, WB120_C], bf16)
            fbB_t = cpool.tile([128, FBB_C], f32)
            fbG_t = cpool.tile([P120, FBG_C], f32)
            fb12_t = cpool.tile([44, FB12_C], f32)
            stat_t = cpool.tile([P120, ST_C], f32)

            dma = nc.sync.dma_start
            dma(out=xp_t[:], in_=xp_d[:])
            dma(out=fbB_t[:], in_=fbB_d[:])
            dma(out=wb12_t[:], in_=wb12_d[:])
            dma(out=fbG_t[:], in_=fbG_d[:])
            dma(out=fb12_t[:], in_=fb12_d[:])
            s60 = 120 + 2 * P124 + 24
            s120 = P124 + 12
            dma(out=wb60_t[:, :s60], in_=wb60_d[:, :s60])
            dma(out=wb120_t[:, :s120], in_=wb120_d[:, :s120])
            dma(out=wb60_t[:, s60:4 * s60], in_=wb60_d[:, s60:4 * s60])
            dma(out=wb120_t[:, s120:5 * s120], in_=wb120_d[:, s120:5 * s120])
            dma(out=wb60_t[:, 4 * s60:7 * s60], in_=wb60_d[:, 4 * s60:7 * s60])
            dma(out=wb120_t[:, 5 * s120:], in_=wb120_d[:, 5 * s120:])
            dma(out=wb60_t[:, 7 * s60:], in_=wb60_d[:, 7 * s60:])

            # vps: rows 0:12 hold gamma1*U1@th1, rows 32:44 hold the
            # gamma2*U2@th2 (+ later gamma3*U3@th3) accumulator; one
            # Square op covers both loss nodes, so rows 12:32 are zeroed
            # once and never written.
            vps_t = vpool.tile([44, F], f32, name="vps")
            nc.vector.memset(vps_t[:], 0.0)

            X = xp_t
            scrq = spool.tile([P12, F], bf16, name="scrq", tag="scrq")
            nc.vector.scalar_tensor_tensor(
                out=scrq[:], in0=X[:], scalar=0.0, in1=X[:],
                op0=OP.add, op1=OP.mult,
                accum_out=stat_t[:P12, 32:33])

            pre31p = None
            for c in range(N_CALLS):
                b12 = 60 + (120 + P124) * c
                b60 = s60 * c
                b61 = b60 + 120 + P124
                b120 = s120 * c
                pre2 = prepool.tile([P120, F], f32, name="pre2", tag="pre2")
                pre31 = prepool.tile([P124, F], f32, name="pre31",
                                     tag="pre31")
                th1 = thpool.tile([P60, F], bf16, name="th1", tag="th1")
                if c == 0:
                    pre1 = prepool.tile([P60, F], f32, name="pre1",
                                        tag="pre1", bufs=1)
                    nc.tensor.matmul(pre1[:], wb12_t[:, 0:60], X[:],
                                     start=True, stop=True)
                    nc.scalar.activation(th1[:], pre1[:], AF.Tanh,
                                         bias=fbB_t[:P60, 10:11])
                else:
                    nc.scalar.activation(th1[:], pre31p[OFF1:, :], AF.Tanh,
                                         bias=fbB_t[OFF1:P124, 10 + c:11 + c])
                nc.tensor.matmul(pre2[:], wb12_t[:, b12:b12 + 120], X[:],
                                 start=True, stop=False)
                nc.tensor.matmul(pre31[:],
                                 wb12_t[:, b12 + 120:b12 + 120 + P124],
                                 X[:], start=True, stop=False)
                # stage 2 (midpoint) — M21@th1 leads the PE queue
                nc.tensor.matmul(pre2[:], wb60_t[:, b60:b60 + 120], th1[:],
                                 start=False, stop=True)
                nc.tensor.matmul(pre31[:],
                                 wb60_t[:, b60 + 120:b60 + 120 + P124],
                                 th1[:], start=False, stop=False)
                nc.tensor.matmul(vps_t[:P12, :],
                                 wb60_t[:, b61 + P124:b61 + P124 + 12],
                                 th1[:], start=True, stop=True)
                scrd = spool.tile([P60, F], bf16, name="scrd1", tag="scrd1")
                nc.vector.scalar_tensor_tensor(
                    out=scrd[:], in0=th1[:], scalar=fbG_t[:P60, c:c + 1],
                    in1=th1[:], op0=OP.mult, op1=OP.mult,
                    accum_out=stat_t[:P60, c:c + 1])
                t1 = spool.tile([P12, F], f32, name="t1", tag="t1")
                nc.vector.tensor_add(t1[:], vps_t[:P12, :], X[:])
                th2 = thpool.tile([P120, F], bf16, name="th2", tag="th2")
                nc.scalar.activation(th2[:], pre2[:], AF.Tanh,
                                     bias=fbB_t[:P120, c:c + 1])
                # stage 3 + next pre1 (stacked in pre31)
                nc.tensor.matmul(pre31[:], wb120_t[:, b120:b120 + P124],
                                 th2[:], start=False, stop=True)
                nc.tensor.matmul(vps_t[32:, :],
                                 wb120_t[:, b120 + P124:b120 + P124 + 12],
                                 th2[:], start=True, stop=True)
                scrd2 = spool.tile([P120, F], bf16, name="scrd2",
                                   tag="scrd2")
                nc.vector.scalar_tensor_tensor(
                    out=scrd2[:], in0=th2[:],
                    scalar=fbG_t[:, 11 + c:12 + c], in1=th2[:],
                    op0=OP.mult, op1=OP.mult,
                    accum_out=stat_t[:, 11 + c:12 + c])
                th3 = thpool.tile([P60, F], bf16, name="th3", tag="th3")
                nc.scalar.activation(th3[:], pre31[:P60, :], AF.Tanh,
                                     bias=fbB_t[:P60, 21 + c:22 + c])
                # both loss nodes in one Square (vps rows 12:32 stay zero);
                # must read vps2 before the U3 matmul accumulates over it
                scrsq = spool.tile([44, F], bf16, name="scrsq", tag="scrsq")
                nc.scalar.activation(scrsq[:], vps_t[:], AF.Square,
                                     bias=fb12_t[:, c:c + 1],
                                     accum_out=stat_t[:44, 21 + c:22 + c])
                nc.tensor.matmul(pre31[:], wb60_t[:, b61:b61 + P124],
                                 th3[:], start=False, stop=True,
                                 skip_group_check=True)
                nc.tensor.matmul(vps_t[32:, :],
                                 wb60_t[:, b61 + P124 + 12:b61 + P124 + 24],
                                 th3[:], start=False, stop=True,
                                 skip_group_check=True)
                Xn = xpool.tile([P12, F], bf16, name="X", tag="X")
                nc.vector.tensor_add(Xn[:], vps_t[32:, :], t1[:])
                X = Xn
                pre31p = pre31

            # final node at t = 1.0
            thf = thpool.tile([P60, F], bf16, name="thf", tag="th1")
            nc.scalar.activation(thf[:], pre31p[OFF1:, :], AF.Tanh,
                                 bias=fbB_t[OFF1:P124, 20:21])
            scrdf = spool.tile([P60, F], bf16, name="scrdf", tag="scrd1")
            nc.vector.scalar_tensor_tensor(
                out=scrdf[:], in0=thf[:], scalar=fbG_t[:P60, 10:11],
                in1=thf[:], op0=OP.mult, op1=OP.mult,
                accum_out=stat_t[:P60, 10:11])
            nc.tensor.matmul(vps_t[:P12, :], wb60_t[:, WB60_C - 12:],
                             thf[:], start=True, stop=True)
            scrsf = spool.tile([44, F], bf16, name="scrsf", tag="scrsq")
            nc.scalar.activation(scrsf[:], vps_t[:], AF.Square,
                                 bias=fb12_t[:, 10:11],
                                 accum_out=stat_t[:44, 31:32])
            scrqn = spool.tile([P12, F], bf16, name="scrqn", tag="scrq")
            nc.vector.scalar_tensor_tensor(
                out=scrqn[:], in0=X[:], scalar=fb12_t[:P12, 11:12], in1=X[:],
                op0=OP.add, op1=OP.mult,
                accum_out=stat_t[:P12, 33:34])

            nc.sync.dma_start(out=stat_d[:], in_=stat_t[:])
    nc.compile()
    return nc


def _const_map(prep):
    import ml_dtypes
    b = ml_dtypes.bfloat16
    return dict(wb12=prep['wb12'].astype(b), wb60=prep['wb60'].astype(b),
                wb120=prep['wb120'].astype(b), fbB=prep['fbB'],
                fbG=prep['fbG'], fb12=prep['fb12'])


def _run_device(prep, x):
    from concourse.bass_utils import run_bass_kernel_spmd
    if 'nc' not in _BASS_CACHE:
        _BASS_CACHE['nc'] = _build_bass()
    nc = _BASS_CACHE['nc']
    consts = _const_map(prep)
    in_maps = []
    for c in range(N_CORES):
        m = dict(consts)
        m['xp'] = _pack_x(x[c * R_CORE:(c + 1) * R_CORE])
        in_maps.append(m)
    trace = bool(_os.environ.get('KERNEL_TRACE'))
    res = run_bass_kernel_spmd(nc, in_maps, list(range(N_CORES)),
                               trace=trace)
    _BASS_CACHE['last_result'] = res
    dstat = np.zeros(N_NODE)
    lstat = np.zeros(N_NODE)
    qstat = np.zeros(2)
    for c in range(N_CORES):
        st = res.results[c]['stat'].astype(np.float64)
        for i in range(11):
            dstat[2 * i] += st[:P60, i].sum()
        for i in range(10):
            dstat[2 * i + 1] += st[:, 11 + i].sum()
        for i in range(10):
            lstat[2 * i] += st[:P12, 21 + i].sum()
            lstat[2 * i + 1] += st[32:44, 21 + i].sum()
        lstat[N_NODE - 1] += st[:P12, 31].sum()
        qstat += st[:P12, 32:34].sum(axis=0)
    return dstat, lstat, qstat


# revision 41
# speedup vs baseline: 1.2023x; 1.0759x over previous
"""Trainium2 Bass kernel for nn_Loss_net_58110907515037.

Computes the ODE-flow loss (loss, loss1, loss_KL, loss_F) over R=8192
samples, data-parallel over 8 NeuronCores (1024 samples/core).

Integrator: classic RK3 (Kutta) with step h=0.1 aligned to the FEM
time-cells of Phi.  Float64 study: RK3 h=0.1 truncation is ~4.1e-3 vs
the 2e-2 gate (RK4 h=0.1 was 1.8e-3); RK3 drops the serial tanh chain
from 41 to 31 and the matmul count from ~15 to ~10 per call, and its
stage values double as the quadrature nodes (k1 = start node value,
k2 = midpoint value), so no extra loss matmuls are needed.

Key structural points (per core, NCHUNK=4 sample chunks on partitions):
  - Grid-node stages (t on the 0.1 grid) have only ONE nonzero Phi
    basis -> 15 live hidden rows/chunk (60 partitions); midpoints have
    two -> 30 rows/chunk (120 partitions).  All weights are shrunk
    accordingly (less LDWEIGHTS + PSUM).
  - pre3 (stage-3 preact) and pre1' (next call's stage-1 preact) share
    their A3@X part: both live stacked in ONE [120,F] PSUM tile, fed by
    single matmuls with [W | W'] stacked weights.
  - X update folds beta terms into host-tracked delta (biases adjust);
    vps PSUM tiles double as loss-node values via the kappa trick.
  - Loss reductions run on GpSimd, div reductions + X update on DVE,
    so ACT does nothing but the 31 critical-path tanh ops.
  - sum_r(v+b)^2 is computed as one stt op: (vps + 2b')*vps summed,
    with the b'^2 correction applied host-side.
"""

import numpy as np
import os as _os

# ---- problem constants (must match the reference) ----
T0, T = 0.0, 1.0
M_, L, HID, D = 10, 3, 5, 3
R_TOTAL = 8192
N_CORES = 8
R_CORE = R_TOTAL // N_CORES          # 1024
NCHUNK = 4
F = R_CORE // NCHUNK                 # 256 free dim
P12 = NCHUNK * D                     # 12 partitions for x/vps tiles
P60 = NCHUNK * 15                    # grid-node th partitions
P120 = NCHUNK * 30                   # midpoint th partitions

HC = 0.1                             # RK3 step (one Phi cell)
N_CALLS = 10
N_NODE = 2 * N_CALLS + 1             # 21 quadrature nodes (0.05 grid)
KAP_E = 6.0 / HC                     # v = kap*vps + be at grid nodes
KAP_O = 3.0 / (2.0 * HC)             # ... at midpoints (gamma = 2h/3)

OFF1 = 64                                      # pre1' partition base (32-al.)
P124 = OFF1 + P60                              # stacked pre3/pre1' tile rows

# bank column layouts
WB12_C = 60 + (120 + P124) * N_CALLS           # [12, 2500]
WB60_C = (120 + 2 * P124 + 24) * N_CALLS + 12  # [60, 3932]
WB120_C = (P124 + 12) * N_CALLS                # [120, 1360]
FBB_C = 31                                     # bias bank [128, 31]
FBG_C = 21                                     # g bank [120, 21]
FB12_C = 12                                    # loss-bias bank [44, 12]
ST_C = 34                                      # stat out [120, 34]


def _phi(t):
    grid = np.linspace(T0, T, M_ + 1)
    s = t - grid
    hh = (T - T0) / M_
    relu = lambda a: np.maximum(a, 0.0)
    return (M_ / (T - T0)) * (relu(s + hh) - 2.0 * relu(s) + relu(s - hh))


def _tconsts(m, W1, b1, W2, b2, G):
    """Per-time-point constants at t = m/20 (float64).

    Returns A [K,3], c [K], U [3,K], g [K], be [3] with K = 15*len(nz):
    rows (nz-basis-idx, l, h).  K=15 at grid nodes, 30 at midpoints.
    """
    ph = _phi(m / 20.0)
    nz = sorted(i for i in np.argsort(-np.abs(ph))[:2] if abs(ph[i]) > 1e-9)
    assert len(nz) == (1 if m % 2 == 0 else 2), (m, ph)
    K = 15 * len(nz)
    A = np.zeros((K, D))
    c = np.zeros(K)
    U = np.zeros((D, K))
    g = np.zeros(K)
    be = np.zeros(D)
    for ii, i in enumerate(nz):
        for l in range(L):
            r0 = ii * (L * HID) + l * HID
            A[r0:r0 + HID, :] = W1[i, l]
            c[r0:r0 + HID] = b1[i, l]
            U[:, r0:r0 + HID] = ph[i] * W2[i, l]
            g[r0:r0 + HID] = ph[i] * G[i, l]
        be += ph[i] * b2[i].sum(axis=0)
    return A, c, U, g, be


def _expT(mat, pin, pout):
    """Block-diag lhsT expansion: mat [pout,pin] per chunk ->
    [NCHUNK*pin, NCHUNK*pout]."""
    W = np.zeros((NCHUNK * pin, NCHUNK * pout))
    for u in range(NCHUNK):
        W[u * pin:(u + 1) * pin, u * pout:(u + 1) * pout] = mat.T
    return W


def _prep(W1, b1, W2, b2):
    """Host-side fold of all device constants (float64 -> banks)."""
    W1 = np.asarray(W1, np.float64)
    b1 = np.asarray(b1, np.float64)
    W2 = np.asarray(W2, np.float64)
    b2 = np.asarray(b2, np.float64)
    G = np.einsum('ildh,ilhd->ilh', W2, W1)   # [11, L, HID]
    h = HC

    wb12 = np.zeros((P12, WB12_C))
    wb60 = np.zeros((P60, WB60_C))
    wb120 = np.zeros((P120, WB120_C))
    fbB = np.zeros((128, FBB_C), np.float32)
    fbG = np.zeros((P120, FBG_C), np.float32)
    fb12 = np.zeros((44, FB12_C), np.float32)
    beta2 = np.zeros(N_NODE)
    gsum = np.zeros(N_NODE)
    kap2 = np.zeros(N_NODE)

    t4 = lambda v: np.tile(v, NCHUNK)
    delta = np.zeros(D)
    for c in range(N_CALLS):
        A1, c1, U1, g1, be1 = _tconsts(2 * c, W1, b1, W2, b2, G)
        A2, c2, U2, g2, be2 = _tconsts(2 * c + 1, W1, b1, W2, b2, G)
        A3, c3, U3, g3, be3 = _tconsts(2 * c + 2, W1, b1, W2, b2, G)
        z4 = lambda p: np.zeros((p, OFF1 - P60))
        if c == 0:
            wb12[:, 0:60] = _expT(A1, D, 15)
            fbB[:P60, 10] = t4(c1 + A1 @ delta)        # call-0 th1 bias
        b0 = 60 + (120 + P124) * c
        wb12[:, b0:b0 + 120] = _expT(A2, D, 30)
        wb12[:, b0 + 120:b0 + 120 + P124] = np.hstack(
            [_expT(A3, D, 15), z4(P12), _expT(A3, D, 15)])
        b0 = (120 + 2 * P124 + 24) * c
        wb60[:, b0:b0 + 120] = _expT((h / 2) * A2 @ U1, 15, 30)
        wb60[:, b0 + 120:b0 + 120 + P124] = np.hstack(
            [_expT(-h * A3 @ U1, 15, 15), z4(P60),
             _expT((h / 6) * A3 @ U1, 15, 15)])
        b1_ = b0 + 120 + P124
        wb60[:, b1_:b1_ + P124] = np.hstack(
            [np.zeros((P60, OFF1)), _expT((h / 6) * A3 @ U3, 15, 15)])
        wb60[:, b1_ + P124:b1_ + P124 + 12] = _expT((h / 6) * U1, 15, D)
        wb60[:, b1_ + P124 + 12:b1_ + P124 + 24] = _expT((h / 6) * U3, 15, D)
        b0 = (P124 + 12) * c
        wb120[:, b0:b0 + P124] = np.hstack(
            [_expT(2 * h * A3 @ U2, 30, 15), z4(P120),
             _expT((2 * h / 3) * A3 @ U2, 30, 15)])
        wb120[:, b0 + P124:b0 + P124 + 12] = _expT((2 * h / 3) * U2, 30, D)
        # biases
        fbB[:P120, c] = t4(c2 + A2 @ (delta + (h / 2) * be1))      # th2
        fbB[:P60, 21 + c] = t4(c3 + A3 @ (delta - h * be1 + 2 * h * be2))
        delta = delta + (h / 6.0) * (be1 + 4.0 * be2 + be3)
        fbB[OFF1:P124, 11 + c] = t4(c3 + A3 @ delta)   # next th1 (rows 64+)
        # node data
        fbG[:P60, c] = t4(g1)
        fbG[:, 11 + c] = t4(g2)
        gsum[2 * c] = g1.sum()
        gsum[2 * c + 1] = g2.sum()
        fb12[:P12, c] = t4(be1 / KAP_E)
        fb12[32:, c] = t4(be2 / KAP_O)
        beta2[2 * c] = (be1 ** 2).sum()
        beta2[2 * c + 1] = (be2 ** 2).sum()
        kap2[2 * c] = KAP_E ** 2
        kap2[2 * c + 1] = KAP_O ** 2

    # final node at t = 1.0 (bias for thf already set as call-9 "next th1")
    Af, cf, Uf, gf, bef = _tconsts(2 * N_CALLS, W1, b1, W2, b2, G)
    wb60[:, WB60_C - 12:] = _expT((h / 6) * Uf, 15, D)
    q = N_NODE - 1
    fbG[:P60, 10] = t4(gf)
    gsum[q] = gf.sum()
    fb12[:P12, 10] = t4(bef / KAP_E)
    beta2[q] = (bef ** 2).sum()
    kap2[q] = KAP_E ** 2

    dN = delta - 1.0                                   # MEAN1 = 1.0
    fb12[:P12, 11] = t4(2.0 * dN)

    w1 = np.ones(N_NODE)
    w1[1:-1:2] = 4.0
    w1[2:-1:2] = 2.0
    wq = w1 * (-(h / 6.0))

    return dict(wb12=wb12, wb60=wb60, wb120=wb120, fbB=fbB, fbG=fbG,
                fb12=fb12, beta2=beta2, gsum=gsum, w1=w1, wq=wq, dN=dN,
                kap2=kap2)


def _combine(prep, dstat, lstat, qstat):
    """Final scalar combine from stat sums (already summed over cores and
    partitions): dstat [21], lstat [21], qstat [2]."""
    R = float(R_TOTAL)
    vsq = prep['kap2'] * lstat                       # sum_r ||v||^2 per node
    loss1 = HC / (6.0 * R) * float(np.dot(prep['w1'], vsq))
    divC = float(np.dot(prep['wq'], prep['gsum'] - dstat / R))
    q0_mean = qstat[0] / R
    qN_mean = (qstat[1] + R * float((prep['dN'] ** 2).sum())) / R
    loss_KL = -0.5 * q0_mean + divC + 0.5 * qN_mean
    loss_F = 0.0
    loss = loss1 + loss_KL + loss_F
    f32 = np.float32
    return f32(loss), f32(loss1), f32(loss_KL), f32(loss_F)


def _pack_x(x_core):
    """[R_CORE, D] -> [P12, F] packed (chunk-major partitions), bf16."""
    import ml_dtypes
    return np.ascontiguousarray(
        x_core.reshape(NCHUNK, F, D).transpose(0, 2, 1).reshape(P12, F)
    ).astype(ml_dtypes.bfloat16)


def _model_core(prep, xp):
    """Numpy f32 mirror of the device program for one core.

    xp: [P12, F] bf16.  Returns dstat [21], lstat [21], qstat [2]
    (summed over partitions)."""
    import ml_dtypes
    bf = ml_dtypes.bfloat16
    f32 = np.float32
    wb12, wb60, wb120 = (prep[k].astype(bf).astype(f32)
                         for k in ('wb12', 'wb60', 'wb120'))
    fbB, fbG, fb12 = prep['fbB'], prep['fbG'], prep['fb12']
    dstat = np.zeros(N_NODE)
    lstat = np.zeros(N_NODE)
    qstat = np.zeros(2)

    def mm(lhsT, rhs):
        return (lhsT.T @ rhs.astype(bf).astype(f32)).astype(f32)

    X = xp.astype(f32)
    qstat[0] = (X * X).sum()
    tanh = lambda p, b: np.tanh(p + b[:, None]).astype(bf).astype(f32)
    pre31p = None
    for c in range(N_CALLS):
        if c == 0:
            pre1 = mm(wb12[:, 0:60], X)
            th1 = tanh(pre1, fbB[:P60, 10])
        else:
            th1 = tanh(pre31p[OFF1:], fbB[OFF1:P124, 10 + c])
        dstat[2 * c] = ((th1 * fbG[:P60, c:c + 1]) * th1).sum()
        b0 = 60 + (120 + P124) * c
        a2 = mm(wb12[:, b0:b0 + 120], X)
        a33 = mm(wb12[:, b0 + 120:b0 + 120 + P124], X)
        b0 = (120 + 2 * P124 + 24) * c
        b1_ = b0 + 120 + P124
        vps1 = mm(wb60[:, b1_ + P124:b1_ + P124 + 12], th1)
        lstat[2 * c] = ((vps1 + fb12[:P12, c:c + 1]) ** 2).sum()
        th2 = tanh(a2 + mm(wb60[:, b0:b0 + 120], th1), fbB[:P120, c])
        dstat[2 * c + 1] = ((th2 * fbG[:, 11 + c:12 + c]) * th2).sum()
        bw = (P124 + 12) * c
        pre31 = a33 + mm(wb60[:, b0 + 120:b0 + 120 + P124], th1) \
            + mm(wb120[:, bw:bw + P124], th2)
        vps23 = mm(wb120[:, bw + P124:bw + P124 + 12], th2)
        lstat[2 * c + 1] = ((vps23 + fb12[32:, c:c + 1]) ** 2).sum()
        th3 = tanh(pre31[:P60], fbB[:P60, 21 + c])
        pre31 = pre31 + mm(wb60[:, b1_:b1_ + P124], th3)
        vps23 = vps23 + mm(wb60[:, b1_ + P124 + 12:b1_ + P124 + 24], th3)
        t1 = X + vps1
        X = (t1 + vps23).astype(bf).astype(f32)
        pre31p = pre31
    thf = tanh(pre31p[OFF1:], fbB[OFF1:P124, 20])
    dstat[N_NODE - 1] = ((thf * fbG[:P60, 10:11]) * thf).sum()
    vpsf = mm(wb60[:, WB60_C - 12:], thf)
    q = N_NODE - 1
    lstat[q] = ((vpsf + fb12[:P12, 10:11]) ** 2).sum()
    qstat[1] = ((X + fb12[:P12, 11:12]) * X).sum()
    return dstat, lstat, qstat


def _run_model(prep, x):
    dstat = np.zeros(N_NODE)
    lstat = np.zeros(N_NODE)
    qstat = np.zeros(2)
    for c in range(N_CORES):
        xp = _pack_x(np.asarray(x[c * R_CORE:(c + 1) * R_CORE], np.float32))
        d, l, q = _model_core(prep, xp)
        dstat += d
        lstat += l
        qstat += q
    return _combine(prep, dstat, lstat, qstat)


def kernel(x, W1, b1, W2, b2):
    prep = _prep(W1, b1, W2, b2)
    if _os.environ.get('KERNEL_NUMPY_MODEL'):
        return _run_model(prep, np.asarray(x, np.float32))
    dstat, lstat, qstat = _run_device(prep, np.asarray(x, np.float32))
    return _combine(prep, dstat, lstat, qstat)


_BASS_CACHE = {}


def _build_bass():
    """Build the Bass/Tile program (shape-only; constants arrive as inputs).

    Engine plan per call (steady state, ~2.5us):
      ACT: th1/th2/th3 tanh only, plus one Square per PAIR of calls that
           covers 4 loss-node vps regions at 32-aligned partition bases
           (it slots into the mm15 wait window, off the tanh chain).
      PE:  M21(pre2 start, th1-dep) ... a2(pre2 stop, Xn-dep with slack);
           MN1(pre31 start) a33 MN2(stop); mmU2 mmU1; mm15(reopens pre31
           for the pre1-next half); mmU3 into its own bank so the Square
           read never blocks the X update.
      DVE: div reductions, X-update adds, Square-sum reduce.
    """
    import concourse.mybir as mybir
    from concourse import tile, bacc

    f32 = mybir.dt.float32
    bf16 = mybir.dt.bfloat16
    AF = mybir.ActivationFunctionType
    OP = mybir.AluOpType

    nc = bacc.Bacc(None, target_bir_lowering=False)
    dp = nc.declare_dram_parameter
    xp_d = dp("xp", [P12, F], bf16, isOutput=False)
    wb12_d = dp("wb12", [P12, WB12_C], bf16, isOutput=False)
    wb60_d = dp("wb60", [P60, WB60_C], bf16, isOutput=False)
    wb120_d = dp("wb120", [P120, WB120_C], bf16, isOutput=False)
    fbB_d = dp("fbB", [128, FBB_C], f32, isOutput=False)
    fbG_d = dp("fbG", [P120, FBG_C], f32, isOutput=False)
    fb12_d = dp("fb12", [44, FB12_C], f32, isOutput=False)
    stat_d = dp("stat", [P120, ST_C], f32, isOutput=True)

    with tile.TileContext(nc) as tc:
        with (
            tc.tile_pool(name="const", bufs=1) as cpool,
            tc.tile_pool(name="state", bufs=2) as xpool,
            tc.tile_pool(name="th", bufs=2) as thpool,
            tc.tile_pool(name="scr", bufs=2) as spool,
            tc.tile_pool(name="pre", bufs=2, space="PSUM") as prepool,
            tc.tile_pool(name="vps", bufs=1, space="PSUM") as vpool,
        ):
            # ACT table preload: dummy tanh+square on a zeroed scrap tile so
            # the ~1.3us ACT_TABLE_LOAD overlaps the weight DMAs.
            warm = cpool.tile([1, 8], f32)
            nc.gpsimd.memset(warm[:], 0.0)
            warm2 = cpool.tile([1, 8], f32)
            nc.scalar.activation(warm2[:], warm[:], AF.Tanh)
            nc.scalar.activation(warm2[:], warm[:], AF.Square)

            xp_t = cpool.tile([P12, F], bf16)
            wb12_t = cpool.tile([P12, WB12_C], bf16)
            wb60_t = cpool.tile([P60, WB60_C], bf16)
            wb120_t = cpool.tile([P120, WB120_C], bf16)
            fbB_t = cpool.tile([128, FBB_C], f32)
            fbG_t = cpool.tile([P120, FBG_C], f32)
            fb12_t = cpool.tile([44, FB12_C], f32)
            stat_t = cpool.tile([P120, ST_C], f32)

            dma = nc.sync.dma_start
            dma(out=xp_t[:], in_=xp_d[:])
            dma(out=fbB_t[:], in_=fbB_d[:])
            dma(out=wb12_t[:], in_=wb12_d[:])
            dma(out=fbG_t[:], in_=fbG_d[:])
            dma(out=fb12_t[:], in_=fb12_d[:])
            s60 = 120 + 2 * P124 + 24
            s120 = P124 + 12
            dma(out=wb60_t[:, :s60], in_=wb60_d[:, :s60])
            dma(out=wb120_t[:, :s120], in_=wb120_d[:, :s120])
            dma(out=wb60_t[:, s60:4 * s60], in_=wb60_d[:, s60:4 * s60])
            dma(out=wb120_t[:, s120:5 * s120], in_=wb120_d[:, s120:5 * s120])
            dma(out=wb60_t[:, 4 * s60:7 * s60], in_=wb60_d[:, 4 * s60:7 * s60])
            dma(out=wb120_t[:, 5 * s120:], in_=wb120_d[:, 5 * s120:])
            dma(out=wb60_t[:, 7 * s60:], in_=wb60_d[:, 7 * s60:])

            # vps regions per pair of calls: call even -> rows 0:12 / 32:44,
            # call odd -> 64:76 / 96:108; rows 12:32, 76:96 stay zero so one
            # Square per pair covers all four loss nodes.
            vps_t = vpool.tile([44, F], f32, name="vps")
            nc.vector.memset(vps_t[:], 0.0)
            vps3_t = vpool.tile([P12, F], f32, name="vps3")

            X = xp_t
            scrq = spool.tile([P12, F], bf16, name="scrq", tag="scrq")
            nc.vector.scalar_tensor_tensor(
                out=scrq[:], in0=X[:], scalar=0.0, in1=X[:],
                op0=OP.add, op1=OP.mult,
                accum_out=stat_t[:P12, 32:33])

            pre31p = None
            t1 = None
            t12 = None
            for c in range(N_CALLS):
                b12 = 60 + (120 + P124) * c
                b60 = s60 * c
                b61 = b60 + 120 + P124
                b120 = s120 * c
                pre2 = prepool.tile([P120, F], f32, name="pre2", tag="pre2")
                pre31 = prepool.tile([P124, F], f32, name="pre31",
                                     tag="pre31")
                th1 = thpool.tile([P60, F], bf16, name="th1", tag="th1")
                if c == 0:
                    pre1 = prepool.tile([P60, F], f32, name="pre1",
                                        tag="pre1", bufs=1)
                    nc.tensor.matmul(pre1[:], wb12_t[:, 0:60], X[:],
                                     start=True, stop=True)
                    nc.scalar.activation(th1[:], pre1[:], AF.Tanh,
                                         bias=fbB_t[:P60, 10:11])
                else:
                    nc.scalar.activation(th1[:], pre31p[OFF1:, :], AF.Tanh,
                                         bias=fbB_t[OFF1:P124, 10 + c:11 + c])
                scrd = spool.tile([P60, F], bf16, name="scrd1", tag="scrd1")
                nc.vector.scalar_tensor_tensor(
                    out=scrd[:], in0=th1[:], scalar=fbG_t[:P60, c:c + 1],
                    in1=th1[:], op0=OP.mult, op1=OP.mult,
                    accum_out=stat_t[:P60, c:c + 1])
                # pre2: the th1-dependent part STARTS the group so it can
                # run during th1->th2; the Xn-dependent A part joins late.
                nc.tensor.matmul(pre2[:], wb60_t[:, b60:b60 + 120], th1[:],
                                 start=True, stop=False)
                nc.tensor.matmul(pre2[:], wb12_t[:, b12:b12 + 120], X[:],
                                 start=False, stop=True)
                th2 = thpool.tile([P120, F], bf16, name="th2", tag="th2")
                nc.scalar.activation(th2[:], pre2[:], AF.Tanh,
                                     bias=fbB_t[:P120, c:c + 1])
                nc.tensor.matmul(pre31[:],
                                 wb60_t[:, b60 + 120:b60 + 120 + P124],
                                 th1[:], start=True, stop=False)
                nc.tensor.matmul(pre31[:],
                                 wb12_t[:, b12 + 120:b12 + 120 + P124],
                                 X[:], start=False, stop=False)
                nc.tensor.matmul(pre31[:], wb120_t[:, b120:b120 + P124],
                                 th2[:], start=False, stop=True)
                th3 = thpool.tile([P60, F], bf16, name="th3", tag="th3")
                nc.scalar.activation(th3[:], pre31[:P60, :], AF.Tanh,
                                     bias=fbB_t[:P60, 21 + c:22 + c])
                nc.tensor.matmul(vps_t[32:, :],
                                 wb120_t[:, b120 + P124:b120 + P124 + 12],
                                 th2[:], start=True, stop=True)
                nc.tensor.matmul(vps_t[:P12, :],
                                 wb60_t[:, b61 + P124:b61 + P124 + 12],
                                 th1[:], start=True, stop=True)
                scrd2 = spool.tile([P120, F], bf16, name="scrd2",
                                   tag="scrd2")
                nc.vector.scalar_tensor_tensor(
                    out=scrd2[:], in0=th2[:],
                    scalar=fbG_t[:, 11 + c:12 + c], in1=th2[:],
                    op0=OP.mult, op1=OP.mult,
                    accum_out=stat_t[:, 11 + c:12 + c])
                t1 = spool.tile([P12, F], f32, name="t1", tag="t1")
                nc.vector.tensor_add(t1[:], vps_t[:P12, :], X[:])
                t12 = spool.tile([P12, F], f32, name="t12", tag="t12")
                nc.vector.tensor_add(t12[:], vps_t[32:, :],
                                     t1[:])
                scrsq = spool.tile([44, F], f32, name="scrsq",
                                   tag="scrsq")
                nc.scalar.activation(scrsq[:], vps_t[:], AF.Square,
                                     bias=fb12_t[:, c:c + 1])
                scrs2 = spool.tile([44, F], bf16, name="scrs2",
                                   tag="scrs2")
                nc.vector.tensor_scalar(
                    out=scrs2[:], in0=scrsq[:], scalar1=1.0,
                    scalar2=0.0, op0=OP.mult, op1=OP.add,
                    accum_out=stat_t[:44, 21 + c:22 + c])
                nc.tensor.matmul(pre31[:], wb60_t[:, b61:b61 + P124],
                                 th3[:], start=False, stop=True,
                                 skip_group_check=True)
                nc.tensor.matmul(vps3_t[:],
                                 wb60_t[:, b61 + P124 + 12:b61 + P124 + 24],
                                 th3[:], start=True, stop=True)
                Xn = xpool.tile([P12, F], bf16, name="X", tag="X")
                nc.vector.tensor_add(Xn[:], vps3_t[:], t12[:])
                X = Xn
                pre31p = pre31

            # final node at t = 1.0
            thf = thpool.tile([P60, F], bf16, name="thf", tag="th1")
            nc.scalar.activation(thf[:], pre31p[OFF1:, :], AF.Tanh,
                                 bias=fbB_t[OFF1:P124, 20:21])
            scrdf = spool.tile([P60, F], bf16, name="scrdf", tag="scrd1")
            nc.vector.scalar_tensor_tensor(
                out=scrdf[:], in0=thf[:], scalar=fbG_t[:P60, 10:11],
                in1=thf[:], op0=OP.mult, op1=OP.mult,
                accum_out=stat_t[:P60, 10:11])
            nc.tensor.matmul(vps_t[:P12, :], wb60_t[:, WB60_C - 12:],
                             thf[:], start=True, stop=True)
            scrsf = spool.tile([44, F], f32, name="scrsf", tag="scrsf")
            nc.scalar.activation(scrsf[:], vps_t[:44, :], AF.Square,
                                 bias=fb12_t[:44, 10:11])
            scrf2 = spool.tile([44, F], bf16, name="scrf2", tag="scrf2")
            nc.vector.tensor_scalar(
                out=scrf2[:], in0=scrsf[:], scalar1=1.0, scalar2=0.0,
                op0=OP.mult, op1=OP.add, accum_out=stat_t[:44, 31:32])
            scrqn = spool.tile([P12, F], bf16, name="scrqn", tag="scrq")
            nc.vector.scalar_tensor_tensor(
                out=scrqn[:], in0=X[:], scalar=fb12_t[:P12, 11:12], in1=X[:],
                op0=OP.add, op1=OP.mult,
                accum_out=stat_t[:P12, 33:34])

            nc.sync.dma_start(out=stat_d[:], in_=stat_t[:])
    nc.compile()
    return nc


def _const_map(prep):
    import ml_dtypes
    b = ml_dtypes.bfloat16
    return dict(wb12=prep['wb12'].astype(b), wb60=prep['wb60'].astype(b),
                wb120=prep['wb120'].astype(b), fbB=prep['fbB'],
                fbG=prep['fbG'], fb12=prep['fb12'])


def _run_device(prep, x):
    from concourse.bass_utils import run_bass_kernel_spmd
    if 'nc' not in _BASS_CACHE:
        _BASS_CACHE['nc'] = _build_bass()
    nc = _BASS_CACHE['nc']
    consts = _const_map(prep)
    in_maps = []
    for c in range(N_CORES):
        m = dict(consts)
        m['xp'] = _pack_x(x[c * R_CORE:(c + 1) * R_CORE])
        in_maps.append(m)
    trace = bool(_os.environ.get('KERNEL_TRACE'))
    res = run_bass_kernel_spmd(nc, in_maps, list(range(N_CORES)),
                               trace=trace)
    _BASS_CACHE['last_result'] = res
    dstat = np.zeros(N_NODE)
    lstat = np.zeros(N_NODE)
    qstat = np.zeros(2)
    for c in range(N_CORES):
        st = res.results[c]['stat'].astype(np.float64)
        for i in range(11):
            dstat[2 * i] += st[:P60, i].sum()
        for i in range(10):
            dstat[2 * i + 1] += st[:, 11 + i].sum()
        for i in range(10):
            lstat[2 * i] += st[:P12, 21 + i].sum()
            lstat[2 * i + 1] += st[32:44, 21 + i].sum()
        lstat[N_NODE - 1] += st[:P12, 31].sum()
        qstat += st[:P12, 32:34].sum(axis=0)
    return dstat, lstat, qstat
